# revision 24
# baseline (speedup 1.0000x reference)
"""Bit2Num dequantization kernel for Trainium2 (8 NeuronCores, SPMD).

Reference op: x [1024, 65536] of {0.0, 1.0} f32, B=4.
  bits = x.reshape(1024, 16384, 4)
  out[b, n] = (8*bits[b,n,0] + 4*bits[b,n,1] + 2*bits[b,n,2] + bits[b,n,3] + 0.5) / 16

Sharding: pure data-parallel over batch — 128 rows per core (= 128 SBUF
partitions). Per core: 32 MiB f32 in + 1 MiB packed uint8 out.

HW model (from NTFF profiles): the 16 SDMA engines/core serialize loads
and stores (no duplex), data packets at ~26.5 GB/s/engine quiet →
span floor = (in+out bytes)/~424 GB/s + ~7.2us fixed entry preamble +
tail. TOTAL DMA BYTES DOMINATE BOTH NOISE BANDS (quiet vs partner-core
contended), so the kernel moves the information-theoretic minimum:
33.56 MiB in + 1.05 MiB out (4 bits per output, nibble-packed). The
remaining lever is keeping every compute engine's busy time UNDER the
~83us quiet load stream. DVE stt throughput VARIES BY MACHINE STATE
run-to-run (~0.61-0.78 elem/cycle measured for identical programs), so
the DVE element budget must be MINIMAL, not merely adequate. This
version leaves the DVE only the irreducible L1 pair-combine (32.8K
elem/partition — 55us busy even in the slow state) and fuses the
entire rest of the tree into the PE:
  psum = 64*y[4t] + 16*y[4t+1] + 4*y[4t+2] + y[4t+3]
via FOUR accumulating matmuls per 512-byte chunk with diagonal
stationaries [64I|16I|4I|I] (a tiny [128,512] host-supplied input);
ACT evicts psum -> uint8 SBUF. Strided (stride-4 bf16) matmul moving
operands work fine on HW. Measured engine busy in a slow-DVE sample:
DVE 55.3 / PE 51.2 / ACT 27.9 vs an 83.3us load span — every engine
under the stream in every machine state.

Per-core kernel, pipelined over 1 MiB column segments of [128, 2048]:
  - Loads on the SP HWDGE ring (nc.sync, plain f32). SWDGE (gpsimd)
    rings serialize the pipeline — do not use them for the stream.
  - BITCAST trick: for x in {0.0f, 1.0f} the high half-word of the f32
    IS its bf16 encoding, so bit i sits at bf16 slot 2i+1 (little-
    endian). All DVE reads are 16-bit. Verified bit-exact on HW.
  - Phase A (30 segments in 15 pair-groups): DVE L1 y = 2*v_even +
    v_odd into a 2-segment y tile; PE packs the group with 4
    accumulating matmuls; ACT copies psum to uint8 (exact: integers
    <= 255) and stores per 1024 B (tiles 0-6) / 512 B (tile 7).
  - Phase B (last 512 B tile, tapered 2048 -> 1024 -> 768 -> 256 cols):
    all on the by-then idle DVE (L2 split + pack per segment) so the
    post-last-load chain carries only a 32-byte pack and avoids the
    PE+ACT latency; one final 512 B store.
  - Stores on the ACT HWDGE ring (qScalarDynamicHW). >= 512 B/partition
    per store is MANDATORY (adjacent sub-512B stores RMW the same SDMA
    granule concurrently and corrupt output — measured).
  - Host unpacks nibbles and applies the exact affine (num+0.5)/16
    during the gather; every value exact in f32.
"""

import numpy as np

import concourse.bacc as bacc
import concourse.bass as bass
import concourse.mybir as mybir
from concourse.bass_utils import run_bass_kernel_spmd
from concourse.tile import TileContext

N_CORES = 8
BATCH = 1024
COLS = 65536
B_BITS = 4
ROWS = BATCH // N_CORES          # 128 rows per core == 128 SBUF partitions
OUT_COLS = COLS // B_BITS        # 16384 groups
PACK_COLS = OUT_COLS // 2        # 8192 packed bytes per row

F32 = mybir.dt.float32
BF16 = mybir.dt.bfloat16
U8 = mybir.dt.uint8
MULT = mybir.AluOpType.mult
ADD = mybir.AluOpType.add

# Load schedule: Phase A streams 30 segments of 2048 f32 cols (1 MiB,
# 8 KiB descriptors) in 15 pair-groups; Phase B tapers the final 4096
# cols as 2048 -> 1024 -> 768 -> 256. Store tiles: 7x 1024 B + 2x 512 B
# (>= 512 B/partition per store is mandatory — SDMA granule).
PHASE_A_SEGS = 30
PHASE_B_SEGS = [2048, 1024, 768, 256]
assert PHASE_A_SEGS * 2048 + sum(PHASE_B_SEGS) == COLS
# PE pack runs per PSUM-bank-sized chunk (512 f32 = one 2 KiB bank).
PE_CHUNK = 512


def _build_nc() -> bass.Bass:
    # Bacc (not plain Bass): its compile() pipeline runs
    # generate_event_semaphores, which splits multi-wait sync conditions —
    # TRN2 DMA instructions accept at most one wait.
    nc = bacc.Bacc(None, target_bir_lowering=False)
    x = nc.dram_tensor("x", [ROWS, COLS], F32, kind="ExternalInput")
    # Stationary weights for the PE pack: [64I | 16I | 4I | I], host-built.
    w = nc.dram_tensor("w", [128, 512], BF16, kind="ExternalInput")
    out = nc.dram_tensor("out", [ROWS, PACK_COLS], U8, kind="ExternalOutput")

    with TileContext(nc) as tc:
        with (
            # bufs=8 on the input pool keeps the load ring ~8 segments
            # ahead of compute; work/out pools keep buffer-recycle waits
            # (store receipts) off the critical path.
            tc.tile_pool(name="xin", bufs=8) as xpool,
            tc.tile_pool(name="wgt", bufs=1) as gpool,
            tc.tile_pool(name="work", bufs=4) as wpool,
            tc.tile_pool(name="oout", bufs=3) as opool,
            tc.tile_pool(name="psum", bufs=4, space=bass.MemorySpace.PSUM) as ppool,
        ):
            wt = gpool.tile([128, 512], BF16, tag="wt")
            # Weight load on the ACT ring — tiny (64 KiB) and off the
            # Sync ring so segment 0's load issues first.
            nc.scalar.dma_start(out=wt[:, :], in_=w[:, :])

            # ---- Phase A: segments 0..29 (30 x 2048 cols) in 15
            # pair-groups. DVE does only L1; the PE packs each group
            # via FOUR accumulating matmuls (fused L2+L3):
            #   psum = 64*y[4t] + 16*y[4t+1] + 4*y[4t+2] + y[4t+3]
            # and ACT evicts psum -> uint8. Tiles 0-6 are 1024 B
            # (2 groups each); tile 7 is 512 B (1 group).
            col = 0
            ot = None
            ot_base = 0
            for g in range(15):
                yt = wpool.tile([ROWS, 2048], BF16, tag="yt")
                for h in range(2):
                    xt = xpool.tile([ROWS, 2048], F32, tag="xt")
                    nc.sync.dma_start(out=xt[:, :], in_=x[:, col:col + 2048])
                    col += 2048
                    xb = xt[:, :].bitcast(BF16).rearrange(
                        "p (i four) -> p i four", four=4
                    )
                    nc.vector.scalar_tensor_tensor(
                        out=yt[:, h * 1024:(h + 1) * 1024],
                        in0=xb[:, :, 1], scalar=2.0, in1=xb[:, :, 3],
                        op0=MULT, op1=ADD,
                    )
                if ot is None:
                    ot_w = 512 if g == 14 else 1024
                    ot = opool.tile([ROWS, ot_w], U8, tag="ot")
                    ot_fill = 0
                yq = yt[:, :].rearrange("p (t four) -> p t four", four=4)
                ps = ppool.tile([ROWS, PE_CHUNK], F32, tag="ps")
                for k in range(4):
                    nc.tensor.matmul(
                        ps[:, :], wt[:, k * 128:(k + 1) * 128], yq[:, :, k],
                        start=(k == 0), stop=(k == 3),
                    )
                nc.scalar.activation(
                    out=ot[:, ot_fill:ot_fill + PE_CHUNK], in_=ps[:, :],
                    func=mybir.ActivationFunctionType.Copy,
                    bias=0.0, scale=1.0,
                )
                ot_fill += PE_CHUNK
                if ot_fill == ot_w:
                    nc.scalar.dma_start(
                        out=out[:, ot_base:ot_base + ot_w], in_=ot[:, :]
                    )
                    ot_base += ot_w
                    ot = None

            # ---- Phase B: the last 512 B tile (cols 61440..65536),
            # tapered 2048 -> 1024 -> 256x4 so the post-last-load chain
            # is tiny. All on the (by now idle) DVE: L2 split + pack
            # per segment, one final 512 B store.
            ze = wpool.tile([ROWS, 512], BF16, tag="ze")
            zo = wpool.tile([ROWS, 512], BF16, tag="zo")
            ot = opool.tile([ROWS, 512], U8, tag="ot")
            z_fill = 0
            # Taper sized for the idle-DVE regime: Phase B's serialized
            # per-segment chains (4 DVE ops each) must drain DURING the
            # last loads' arrival window, so FEWER, BIGGER segments beat
            # many tiny ones (a 256x4 taper measured 3.2us of post-last-
            # load op drain from per-op semaphore overhead alone).
            for seg_c in PHASE_B_SEGS:
                xt = xpool.tile([ROWS, seg_c], F32, tag="xt")
                nc.sync.dma_start(out=xt[:, :], in_=x[:, col:col + seg_c])
                col += seg_c
                xb = xt[:, :].bitcast(BF16).rearrange(
                    "p (i four) -> p i four", four=4
                )
                yt = wpool.tile([ROWS, seg_c // 2], BF16, tag="yb")
                nc.vector.scalar_tensor_tensor(
                    out=yt[:, :], in0=xb[:, :, 1], scalar=2.0, in1=xb[:, :, 3],
                    op0=MULT, op1=ADD,
                )
                seg_p = seg_c // 8
                yv = yt[:, :].rearrange("p (t four) -> p t four", four=4)
                nc.vector.scalar_tensor_tensor(
                    out=ze[:, z_fill:z_fill + seg_p],
                    in0=yv[:, :, 0], scalar=4.0, in1=yv[:, :, 1],
                    op0=MULT, op1=ADD,
                )
                nc.vector.scalar_tensor_tensor(
                    out=zo[:, z_fill:z_fill + seg_p],
                    in0=yv[:, :, 2], scalar=4.0, in1=yv[:, :, 3],
                    op0=MULT, op1=ADD,
                )
                nc.vector.scalar_tensor_tensor(
                    out=ot[:, z_fill:z_fill + seg_p],
                    in0=ze[:, z_fill:z_fill + seg_p], scalar=16.0,
                    in1=zo[:, z_fill:z_fill + seg_p], op0=MULT, op1=ADD,
                )
                z_fill += seg_p
            assert z_fill == 512 and col == COLS
            nc.scalar.dma_start(
                out=out[:, ot_base:ot_base + 512], in_=ot[:, :]
            )
            assert ot_base + 512 == PACK_COLS
    # Bacc.finalize runs the compile pipeline (register allocation +
    # generate_event_semaphores); the pjrt exec path serializes nc.m as-is.
    nc.finalize()
    return nc


_NC = None


def _get_nc() -> bass.Bass:
    global _NC
    if _NC is None:
        _NC = _build_nc()
    return _NC


def _make_w() -> np.ndarray:
    import ml_dtypes

    eye = np.eye(128, dtype=np.float32)
    return np.concatenate(
        [64.0 * eye, 16.0 * eye, 4.0 * eye, eye], axis=1
    ).astype(ml_dtypes.bfloat16)


def make_in_maps(x: np.ndarray) -> list[dict]:
    w = _make_w()
    return [
        {"x": x[i * ROWS:(i + 1) * ROWS], "w": w} for i in range(N_CORES)
    ]


def kernel(x: np.ndarray, B=4) -> np.ndarray:
    assert int(B) == B_BITS, f"kernel hardcodes B={B_BITS}, got {B}"
    x = np.ascontiguousarray(x, dtype=np.float32)
    assert x.shape == (BATCH, COLS), x.shape
    nc = _get_nc()
    res = run_bass_kernel_spmd(nc, make_in_maps(x), list(range(N_CORES)))
    packed = np.concatenate(
        [res.results[i]["out"] for i in range(N_CORES)], axis=0
    )
    # Unpack nibbles (group 2j in the high nibble) and apply the exact
    # affine (num + 0.5) / 16 on the host — every value exact in f32.
    res_f = np.empty((BATCH, OUT_COLS), dtype=np.float32)
    res_f[:, 0::2] = (packed >> 4).astype(np.float32)
    res_f[:, 1::2] = (packed & 15).astype(np.float32)
    res_f += np.float32(0.5)
    res_f *= np.float32(1.0 / 16.0)
    return res_f
